# revision 26
# baseline (speedup 1.0000x reference)
"""Encoder-decoder attention kernel for Trainium2, 8 NeuronCores.

Sharding: batch (B=8) data-parallel, one batch element per core; weights
replicated. Per core (S=Sq=Sk=1024, H=1024, NH=16, D=64):

  phase A: transpose X_dec, X_enc via PE (fp32) -> xdt/xet [128, 8h, 1024s]
  prep (JIT per pair, woven into attention slots):
    Q^T[p] = (Wq/8 slice)^T @ X_dec^T   -- f32r matmuls (1 cyc/row, ~fp32)
    K^T[p] likewise; V[k, nd] = X_enc^T-slices @ Wv -- f32r -> fp16 v2
  per (pair, qtile): scores S = Q'^T K, two K=64 f32r matmuls row-tiled at
    partitions 0/64 (concurrent in the PE array), N=512 chunks
  softmax: DVE negated-max reduce -> ACT exp(bias=-max, accum_out=rowsum
    -> Rsrc) -> fp16 p_e (unnormalized); P^T via one DMA transpose/tile
  normalization is folded PAST the PV matmul: O^T columns are scaled by
    1/rowsum during the PSUM->concat copy (16x fewer elements than
    scaling P).  R tiles are built per 2-qtile block: batched reciprocal,
    PE transpose of [128,4] recips, DMA partition-broadcast.
  PV: O^T = V^T P^T fp16, two M=64 matmuls col-tiled at (0,0)/(0,64)
  phase D: out = concat^T @ W_out^T + b (fp16 weights from host) at end

Precision: q/k/scores in f32r (~1.5e-4 rounding); scores are ~N(0,341)
but softmax is extremely peaked (top-2 gaps ~100), so the resulting
softmax-weight perturbations stay well inside the 2e-2 gate. P/V in
fp16; output projection fp16. 1/sqrt(D)=1/8 folded into W_query on host.
"""
import sys

sys.path.insert(0, "/opt/trn_rl_repo")

import numpy as np

B = 8
S = 1024   # Sq == Sk
H = 1024
NH = 16
D = 64
P = 128
HT = H // P    # 8 h-tiles
ST = S // P    # 8 s-tiles == k-tiles
NP = NH // 2   # 8 head pairs
QB = 256       # q-block width for the P@V moving dim
NB = S // QB   # 4 q-blocks per pair
QTB = QB // P  # 2 q-tiles per block


def build():
    import concourse.mybir as mybir
    import concourse.tile as tile
    from concourse import bacc
    from concourse.masks import make_identity

    f32 = mybir.dt.float32
    f32r = mybir.dt.float32r
    f16 = mybir.dt.float16
    AX = mybir.AxisListType.X
    OP = mybir.AluOpType
    AF = mybir.ActivationFunctionType

    nc = bacc.Bacc(trn_type="TRN2", target_bir_lowering=False, debug=False)

    xd_d = nc.dram_tensor("xd", [S, H], f32, kind="ExternalInput").ap()
    xe_d = nc.dram_tensor("xe", [S, H], f32, kind="ExternalInput").ap()
    # wq4/wk4: [pair, h_local(128), (j, nd_local)] -- per-pair contiguous
    wq_d = nc.dram_tensor("wq4", [NP, P, H], f32r, kind="ExternalInput").ap()
    wk_d = nc.dram_tensor("wk4", [NP, P, H], f32r, kind="ExternalInput").ap()
    # wv4: [grp, h_local(128), (j, nd512)]
    wv_d = nc.dram_tensor("wv4", [2, P, HT * 512], f32r, kind="ExternalInput").ap()
    # wo2: [half, nd_local(128), (pair, h512)] fp16
    wo_d = nc.dram_tensor("wo2", [2, P, NP * 512], f16, kind="ExternalInput").ap()
    bias_d = nc.dram_tensor("bias", [P, H], f32, kind="ExternalInput").ap()
    out_d = nc.dram_tensor("out", [S, H], f32, kind="ExternalOutput").ap()

    def r(ap):
        return ap.bitcast(f32r)

    from contextlib import ExitStack
    with tile.TileContext(nc) as tc:
        with ExitStack() as ctx:
            permp = ctx.enter_context(tc.tile_pool(name="perm", bufs=2))
            xinp = ctx.enter_context(tc.tile_pool(name="xin", bufs=2))
            wqp = ctx.enter_context(tc.tile_pool(name="wq", bufs=2))
            wkp = ctx.enter_context(tc.tile_pool(name="wk", bufs=2))
            wvp = ctx.enter_context(tc.tile_pool(name="wv", bufs=2))
            qtp = ctx.enter_context(tc.tile_pool(name="qt", bufs=2))
            ktp = ctx.enter_context(tc.tile_pool(name="kt", bufs=2))
            vpp = ctx.enter_context(tc.tile_pool(name="vp", bufs=2))
            ccp = ctx.enter_context(tc.tile_pool(name="cc", bufs=NP))
            pep = ctx.enter_context(tc.tile_pool(name="pe", bufs=2))
            ptp = ctx.enter_context(tc.tile_pool(name="pt", bufs=4))
            rsp = ctx.enter_context(tc.tile_pool(name="rs", bufs=2))
            rrp = ctx.enter_context(tc.tile_pool(name="rr", bufs=2))
            rtp = ctx.enter_context(tc.tile_pool(name="rt", bufs=2))
            rbp = ctx.enter_context(tc.tile_pool(name="rb", bufs=2))
            wop = ctx.enter_context(tc.tile_pool(name="wo", bufs=2))
            osbp = ctx.enter_context(tc.tile_pool(name="osb", bufs=1))
            constp = ctx.enter_context(tc.tile_pool(name="const", bufs=1))
            statp = ctx.enter_context(tc.tile_pool(name="stat", bufs=16))
            psp = ctx.enter_context(tc.tile_pool(name="ps", bufs=2, space="PSUM"))
            psSp = ctx.enter_context(tc.tile_pool(name="psS", bufs=3, space="PSUM"))

            def pstile():
                return psp.tile([P, 512], f32, tag="ps", name="ps")

            def pstileS():
                return psSp.tile([P, S], f32, tag="psS", name="psS")

            def stat():
                return statp.tile([P, 1], f32, tag="stat", name="stat")

            # ---- constants ----
            ident = constp.tile([P, P], f32)
            make_identity(nc, ident[:])
            bias_sb = constp.tile([P, H], f32)
            nc.scalar.dma_start(bias_sb[:], bias_d)
            # warmup transpose absorbs the gpsimd(identity) dep on PE
            warm = pstile()
            nc.tensor.transpose(warm[:, 0:P], ident[:], ident[:])
            # HAM warm-up: ~5us of continuous junk matmuls while the
            # input DMAs land, so phase A + the first chunks run at
            # 2.4 GHz instead of 1.2 (the activity window needs ~3.4us
            # of sustained matmul work; transposes don't count)
            junkw = constp.tile([P, 512], mybir.dt.bfloat16)
            nc.vector.memset(junkw[:], 0.5)
            junk_ps = pstileS()
            for _ in range(26):
                nc.tensor.matmul(junk_ps[:, 0:512], junkw[:, 0:P],
                                 junkw[:], start=True, stop=True)

            # wo halves up front (fp16, host-prelaid)
            wo_sb = []
            for half in range(2):
                w = wop.tile([P, NP * 512], f16, tag="wo", name="wo")
                nc.gpsimd.dma_start(w[:], wo_d[half])
                wo_sb.append(w)

            # ---- phase A: X^T via PE transposes ----
            xdt = permp.tile([P, HT, S], f32r, tag="perm", name="xdt")
            xet = permp.tile([P, HT, S], f32r, tag="perm", name="xet")
            for lst, src in ((xdt, xd_d), (xet, xe_d)):
                for i in range(ST):
                    xin = xinp.tile([P, H], f32, tag="xin")
                    nc.gpsimd.dma_start(xin[:], src[i * P:(i + 1) * P, :])
                    for g in range(2):
                        pst = pstile()
                        for t in range(4):
                            j = g * 4 + t
                            nc.tensor.transpose(
                                pst[:, t * P:(t + 1) * P],
                                xin[:, j * P:(j + 1) * P], ident[:])
                        nc.vector.tensor_copy(
                            lst[:, g * 4:(g + 1) * 4, i * P:(i + 1) * P],
                            pst[:].rearrange("p (a b) -> p a b", a=4))

            # ---- prep chunks ----
            q_t = {}
            k_next = [None]
            v2_next = [None]

            def qk_chunk(p, which):
                # load() fires the weight DMA at pair start -- ahead of
                # the pair's R-broadcast sem-waits in the in-order gpsimd
                # queue, so the chunk's first LDWEIGHTS never stalls.
                # The 16 matmuls of compute() stay contiguous on the PE
                # queue (interleaving foreign matmuls inside an open PSUM
                # accumulation group corrupts results).
                box = {}

                def load():
                    wsb_pool, w_d = ((wqp, wq_d) if which == "q"
                                     else (wkp, wk_d))
                    wsb = wsb_pool.tile([P, HT, P], f32r, tag="w", name="w")
                    nc.gpsimd.dma_start(
                        wsb[:].rearrange("p a b -> p (a b)"), w_d[p])
                    box["w"] = wsb

                def compute_half(nn):
                    def c():
                        wsb = box["w"]
                        ps1 = pstile()
                        xt = xdt if which == "q" else xet
                        sl = slice(nn * 512, (nn + 1) * 512)
                        for j in range(HT):
                            nc.tensor.matmul(ps1[:], wsb[:, j, :],
                                             xt[:, j, sl],
                                             start=(j == 0),
                                             stop=(j == HT - 1))
                        if nn == 0:
                            dstp = qtp if which == "q" else ktp
                            box["d"] = dstp.tile([P, S], f32r, tag="d",
                                                 name=which)
                        dst = box["d"]
                        nc.scalar.copy(dst[:, sl], ps1[:])
                        if nn == 1:
                            if which == "q":
                                q_t[p] = dst
                            else:
                                k_next[0] = dst
                    return c

                return load, [compute_half(0), compute_half(1)]

            def v_chunks(grp):
                v2_box = [None]
                wv_box = {}

                def load(g):
                    def ld():
                        wvh = wvp.tile([P, 4, 512], f32r, tag="wv", name="wv")
                        nc.gpsimd.dma_start(
                            wvh[:].rearrange("p a b -> p (a b)"),
                            wv_d[grp][:, g * 2048:(g + 1) * 2048])
                        wv_box[g] = wvh
                    return ld

                def mk(g, kk):
                    def cg():
                        if v2_box[0] is None:
                            v2_box[0] = vpp.tile([P, ST, 512], f16, tag="vp",
                                                 name="v2")
                            v2_next[0] = v2_box[0]
                        v2n = v2_box[0]
                        ps1 = pstile()
                        kt = g * 2 + kk
                        for j in range(HT):
                            nc.tensor.matmul(
                                ps1[:], xet[:, j, kt * P:(kt + 1) * P],
                                wv_box[j // 4][:, j % 4, :],
                                start=(j == 0), stop=(j == HT - 1))
                        nc.scalar.copy(v2n[:, kt, :], ps1[:])
                    return cg

                return [load(0), load(1)], [mk(g, kk) for g in range(4) for kk in range(2)]

            # prologue: Q/K for pair 0, V groups g0-g1 for pairs 0-3
            for which in ("q", "k"):
                ld, comp = qk_chunk(0, which)
                ld()
                for c in comp:
                    c()
            vld0, vch0 = v_chunks(0)
            vld0[0]()
            vld0[1]()
            for c in vch0[0:6]:
                c()
            k_s = k_next[0]
            v2 = v2_next[0]

            # ---- main pair loop ----
            concat_t = []
            vch_cache = {1: None}
            pending_pv = []

            def emit_pv(args):
                v2_, vc_, pt_ev_, pt_od_, concat_, blk_, rblk_ = args
                ps_o = pstile()
                for kt_i in range(ST):
                    nc.tensor.matmul(
                        ps_o[0:64, 0:QB],
                        v2_[:, kt_i, vc_:vc_ + 64],
                        pt_ev_[:, kt_i, :],
                        start=(kt_i == 0), stop=(kt_i == ST - 1),
                        tile_position=(0, 0))
                    nc.tensor.matmul(
                        ps_o[64:128, 0:QB],
                        v2_[:, kt_i, vc_ + 64:vc_ + 128],
                        pt_od_[:, kt_i, :],
                        start=(kt_i == 0), stop=(kt_i == ST - 1),
                        tile_position=(0, 64))
                # rblk holds [h0-recips | h1-recips] along free dim on all
                # partitions; each 64-row half reads its own 256-col slice
                nc.vector.tensor_tensor(
                    concat_[0:64, blk_ * QB:(blk_ + 1) * QB],
                    ps_o[0:64, 0:QB], rblk_[0:64, 0:QB], op=OP.mult)
                nc.vector.tensor_tensor(
                    concat_[64:128, blk_ * QB:(blk_ + 1) * QB],
                    ps_o[64:128, 0:QB], rblk_[64:128, QB:2 * QB],
                    op=OP.mult)

            rb_queue = []

            def run_rbuild(args):
                # R for a block: recip -> transposes -> broadcast.  Runs
                # one qt AFTER the block's last exp so the PE-queue
                # transposes never wait on the softmax chain.
                Rsrc_, pvargs = args
                Rrec = rrp.tile([P, 2 * QTB], f32, tag="rr", name="rrec")
                nc.vector.reciprocal(Rrec[:], Rsrc_[:])
                # [128,1] transposes land every recip in partition 0,
                # free = (h01, qtl, ql); broadcast to all 128 partitions
                ps_rt = pstile()
                for c in range(2 * QTB):
                    h01_, qtl_ = c % 2, c // 2
                    off = (h01_ * QTB + qtl_) * P
                    nc.tensor.transpose(
                        ps_rt[0:1, off:off + P],
                        Rrec[:, c:c + 1], ident[:])
                rtr = rtp.tile([1, 2 * QTB * P], f32, tag="rt", name="rtr")
                nc.scalar.copy(rtr[:], ps_rt[0:1, 0:2 * QTB * P])
                rblk = rbp.tile([P, 2 * QB], f32, tag="rb", name="rb")
                nc.gpsimd.partition_broadcast(rblk[:], rtr[0:1], channels=P)
                pending_pv.append(pvargs + (rblk,))

            for p in range(NP):
                # V chunks lead: PV matmuls of a pair sit earlier in the
                # in-order PE queue than later pairs' chunks, so all of a
                # group's V work must be issued before the first PV that
                # reads it (pair 0 needs g2/g3 of grp0; pair 4 needs all
                # of grp1 -> grp1 spread over pairs 1-3).
                chunks = []
                loads = []
                if p == 0:
                    chunks += vch0[6:8]
                elif p < 4:
                    if vch_cache[1] is None:
                        vch_cache[1] = v_chunks(1)
                    vloads, vv = vch_cache[1]
                    if p == 1:
                        loads += vloads
                    chunks += (vv[2 * (p - 1):2 * p] if p < 3
                               else vv[4:8])
                if p + 1 < NP:
                    for which in ("k", "q"):
                        ld, comp = qk_chunk(p + 1, which)
                        loads.append(ld)
                        chunks += comp
                # fire this pair's weight DMAs now, before any R
                # broadcasts enter the gpsimd queue
                for ld in loads:
                    ld()
                vc = (p % 4) * P

                concat = ccp.tile([P, S], f16, tag="cc", name="concat")
                concat_t.append(concat)

                def slot():
                    # one unit of PE filler per score tile; when no real
                    # work is ready, a short junk burst keeps the PE's
                    # HAM activity window warm (idle >~3us drops the
                    # clock to 1.2 GHz for everything that follows)
                    if pending_pv:
                        emit_pv(pending_pv.pop(0))
                    elif chunks:
                        chunks.pop(0)()
                    else:
                        jp = pstile()
                        for _ in range(4):
                            nc.tensor.matmul(jp[:], junkw[:, 0:P],
                                             junkw[:], start=True,
                                             stop=True)

                pt_ev = pt_od = Rsrc = None
                for qt in range(ST):
                    qtl = qt % QTB
                    blk = qt // QTB
                    if qtl == 0:
                        pt_ev = ptp.tile([P, ST, QB], f16, tag="pt",
                                         name="ptev")
                        pt_od = ptp.tile([P, ST, QB], f16, tag="pt",
                                         name="ptod")
                        Rsrc = rsp.tile([P, 2 * QTB], f32, tag="rs",
                                        name="rsrc")
                    ps_s = [pstileS(), pstileS()]
                    for h01 in range(2):
                        hs = slice(h01 * 64, (h01 + 1) * 64)
                        qstat = q_t[p][hs, qt * P:(qt + 1) * P]
                        for kk in range(2):
                            ks = slice(kk * 512, (kk + 1) * 512)
                            nc.tensor.matmul(
                                ps_s[h01][:, ks], qstat,
                                k_s[hs, ks], start=True, stop=True)
                    for h01 in range(2):
                        negmax = stat()
                        nc.vector.tensor_reduce(
                            negmax[:], ps_s[h01][:], axis=AX,
                            op=OP.max, negate=True)
                        p_e = pep.tile([P, S], f16, tag="pe")
                        col = qtl * 2 + h01
                        nc.scalar.activation(
                            p_e[:], ps_s[h01][:], AF.Exp,
                            bias=negmax[:],
                            accum_out=Rsrc[:, col:col + 1])
                        pt_dst = pt_ev if h01 == 0 else pt_od
                        nc.sync.dma_start_transpose(
                            pt_dst[:, :, qtl * P:(qtl + 1) * P], p_e[:])
                    # PE filler after the exps (ACT runs in issue order)
                    slot()
                    if rb_queue:
                        run_rbuild(rb_queue.pop(0))
                    if qtl == QTB - 1:
                        rb_queue.append(
                            (Rsrc, (v2, vc, pt_ev, pt_od, concat, blk)))
                # end-of-pair drain: leftover chunks + any R builds;
                # pending PVs carry into the next pair
                for c in chunks:
                    c()
                while rb_queue:
                    run_rbuild(rb_queue.pop(0))
                if p + 1 < NP:
                    k_s = k_next[0]
                    if p == 3:
                        v2 = v2_next[0]
            while pending_pv:
                emit_pv(pending_pv.pop(0))

            # ---- phase D: out = concat^T @ W_out^T + b ----
            for sg in range(2):
                ps_big = [pstileS(), pstileS(), pstileS()]
                ps_sm = [pstile(), pstile()]

                def out_slot(sl, half):
                    if sl < 3:
                        return ps_big[sl][:, half * 512:(half + 1) * 512]
                    return ps_sm[half][:]

                for p in range(NP):
                    for sl in range(4):
                        st = sg * 4 + sl
                        stat_ = concat_t[p][:, st * P:(st + 1) * P]
                        for half in range(2):
                            nc.tensor.matmul(
                                out_slot(sl, half), stat_,
                                wo_sb[half][:, p * 512:(p + 1) * 512],
                                start=(p == 0), stop=(p == NP - 1))
                for sl in range(4):
                    st = sg * 4 + sl
                    osb = osbp.tile([P, H], f32, tag="osb")
                    if sl < 3:
                        nc.vector.tensor_tensor(
                            osb[:], ps_big[sl][:], bias_sb[:], op=OP.add)
                    else:
                        for half in range(2):
                            nc.vector.tensor_tensor(
                                osb[:, half * 512:(half + 1) * 512],
                                ps_sm[half][:],
                                bias_sb[:, half * 512:(half + 1) * 512],
                                op=OP.add)
                    nc.scalar.dma_start(out_d[st * P:(st + 1) * P, :], osb[:])

    nc.compile()
    return nc


def prep_in_maps(decoder_input, encoder_output, W_query, W_key, W_value,
                 W_out, b_out):
    f = lambda a: np.ascontiguousarray(np.asarray(a, dtype=np.float32))
    di = f(decoder_input)
    eo = f(encoder_output)
    wqT = (f(W_query).reshape(H, H) * np.float32(0.125)).T  # [h, nd]
    wkT = f(W_key).reshape(H, H).T
    wvT = f(W_value).reshape(H, H).T
    woT = f(W_out).T                                        # [nd, h_out]
    wq4 = np.ascontiguousarray(
        wqT.reshape(HT, P, NP, P).transpose(2, 1, 0, 3).reshape(NP, P, H))
    wk4 = np.ascontiguousarray(
        wkT.reshape(HT, P, NP, P).transpose(2, 1, 0, 3).reshape(NP, P, H))
    wv4 = np.ascontiguousarray(
        wvT.reshape(HT, P, 2, 512).transpose(2, 1, 0, 3).reshape(2, P, HT * 512))
    wo2 = np.ascontiguousarray(
        woT.reshape(NP, P, 2, 512).transpose(2, 1, 0, 3)
        .reshape(2, P, NP * 512).astype(np.float16))
    bias = np.ascontiguousarray(np.broadcast_to(f(b_out), (P, H)))
    return [
        {"xd": di[b], "xe": eo[b], "wq4": wq4, "wk4": wk4, "wv4": wv4,
         "wo2": wo2, "bias": bias}
        for b in range(B)
    ]


_BUILT = None


def kernel(decoder_input, encoder_output, W_query, W_key, W_value, W_out,
           b_out):
    global _BUILT
    from concourse import bass_utils
    if _BUILT is None:
        _BUILT = build()
    in_maps = prep_in_maps(decoder_input, encoder_output, W_query, W_key,
                           W_value, W_out, b_out)
    try:
        res = bass_utils.run_bass_kernel_spmd(_BUILT, in_maps,
                                              core_ids=list(range(B)))
    except Exception:
        # one retry: a previously wedged NeuronCore can fail the first
        # execution after load
        res = bass_utils.run_bass_kernel_spmd(_BUILT, in_maps,
                                              core_ids=list(range(B)))
    return np.stack([res.results[b]["out"] for b in range(B)], axis=0)
